# revision 1
# baseline (speedup 1.0000x reference)
"""AttentionConv2d pooling kernel for 8 Trainium2 NeuronCores.

Math: the reference computes, per batch n:
    tok = x[n].reshape(D, L).T                      # [L, D]
    K   = tok @ k_w.T + k_b + pos                   # [L, DOUT]
    V   = tok @ v_w.T + v_b                         # [L, DOUT]
    s   = K @ query / sqrt(DOUT)                    # [L]
    a   = softmax(s)                                # [L]
    out = a @ V                                     # [DOUT]

which collapses (since sum(a) == 1) to:
    q'  = k_w.T @ query / sqrt(DOUT)                # [D]
    ps  = (posMLP(grid) @ query) / sqrt(DOUT)       # [L]   (fourier MLP)
    c   = (k_b + b2) @ query / sqrt(DOUT)           # scalar, via Exp bias
    s   = x[n].T @ q' + ps
    u   = exp(s + c)    (scores are O(5), no max-subtraction needed)
    w   = x[n] @ u / sum(u)                         # [D]
    out = w @ v_w.T + v_b                           # [DOUT]

Sharding: data-parallel over batch N (2 batches per core); the fourier-MLP
pos-score is sharded over L: each core computes only its own l-range and the
8 ranges are exchanged with an AllGather.

Rotation trick: the host rotates each core's x chunks so that chunk j holds
l-range (c+j)%8 (softmax sums are order-invariant). Chunk 0 is then the
core's OWN range, whose pos row is computed locally anyway for the gather —
so the main loop can start before the collective lands, with no redundant
pos-MLP work. Chunks 1..7 take their pos row from the gathered [8, LSH]
table via a one-hot selector matmul whose selector matrix is per-core INPUT
DATA (the compiled program stays identical across cores).
"""

import contextlib
import ctypes
import sys
import types

import numpy as np

# ---------------------------------------------------------------------------
# antenv.axon_hooks shim: the image lacks this module; bass_utils imports it
# to capture NTFF profiles when trace=True. Provide the ctypes equivalent.
# ---------------------------------------------------------------------------
if "antenv.axon_hooks" not in sys.modules:
    _HOOK_CACHE = []

    def _make_ntff_hook():
        try:
            lib = ctypes.CDLL("/opt/axon/libaxon_pjrt.so")
        except OSError:
            return None
        if not hasattr(lib, "axon_start_nrt_profile"):
            return None
        lib.axon_start_nrt_profile.argtypes = [
            ctypes.POINTER(ctypes.c_int64),
            ctypes.c_size_t,
        ]
        lib.axon_start_nrt_profile.restype = ctypes.c_int64
        lib.axon_stop_nrt_profile.argtypes = [ctypes.c_char_p]
        lib.axon_stop_nrt_profile.restype = ctypes.c_int64

        @contextlib.contextmanager
        def _hook(output_dir, device_ids):
            import jax

            jax.devices()
            if device_ids:
                ids = (ctypes.c_int64 * len(device_ids))(*device_ids)
                rc = lib.axon_start_nrt_profile(ids, len(device_ids))
            else:
                rc = lib.axon_start_nrt_profile(None, 0)
            if rc != 0:
                raise RuntimeError(f"axon_start_nrt_profile rc={rc}")
            try:
                yield
            finally:
                n = lib.axon_stop_nrt_profile(str(output_dir).encode())
                print(f"ntff profile: {n} file(s) written to {output_dir}")

        return _hook

    def get_axon_ntff_profile_hook():
        if not _HOOK_CACHE:
            _HOOK_CACHE.append(_make_ntff_hook())
        return _HOOK_CACHE[0]

    _mod = types.ModuleType("antenv.axon_hooks")
    _mod.get_axon_ntff_profile_hook = get_axon_ntff_profile_hook
    sys.modules["antenv.axon_hooks"] = _mod

import concourse.bass as bass  # noqa: E402
import concourse.mybir as mybir  # noqa: E402
import concourse.tile as tile  # noqa: E402
from concourse import bacc  # noqa: E402
from concourse.bass_utils import run_bass_kernel_spmd  # noqa: E402
from concourse.masks import make_identity  # noqa: E402

# Problem shapes (hardcoded per spec).
N, D, H, W = 16, 256, 128, 128
L = H * W  # 16384
DOUT = 256
NCORES = 8
NB = N // NCORES  # batches per core = 2
LSH = L // NCORES  # pos-score shard per core = 2048
LC = 2048  # l-chunk for the main loop (== LSH)
NCHUNK = L // LC  # chunks per batch = 8

F32 = mybir.dt.float32
F32R = mybir.dt.float32r
AF = mybir.ActivationFunctionType
OP = mybir.AluOpType

INV_SQRT_D = 1.0 / 16.0  # 1/sqrt(DOUT)
HALF_PI = float(np.pi / 2.0)


def _r(ap):
    """Bitcast an fp32 AP to fp32r (fp22-truncated full-rate PE matmuls)."""
    return ap.bitcast(F32R)


def build_program():
    nc = bacc.Bacc(
        "TRN2",
        target_bir_lowering=False,
        debug=False,
        enable_asserts=True,
        num_devices=NCORES,
    )

    # Per-core DRAM I/O. x_sh is this core's batch shard with the l-chunks
    # rotated so chunk j is l-range (c+j)%8; gg is this core's own [gy; gx]
    # grid rows; sel is the per-chunk one-hot range selector.
    x_d = nc.dram_tensor("x_sh", [NB, D, L], F32, kind="ExternalInput").ap()
    query_d = nc.dram_tensor("query", [DOUT], F32, kind="ExternalInput").ap()
    kw_d = nc.dram_tensor("k_w", [DOUT, D], F32, kind="ExternalInput").ap()
    kb_d = nc.dram_tensor("k_b", [DOUT], F32, kind="ExternalInput").ap()
    vw_d = nc.dram_tensor("v_w", [DOUT, D], F32, kind="ExternalInput").ap()
    vb_d = nc.dram_tensor("v_b", [DOUT], F32, kind="ExternalInput").ap()
    wr_d = nc.dram_tensor("Wr", [DOUT // 2, 2], F32, kind="ExternalInput").ap()
    w1_d = nc.dram_tensor("w1", [DOUT, DOUT], F32, kind="ExternalInput").ap()
    b1_d = nc.dram_tensor("b1", [DOUT], F32, kind="ExternalInput").ap()
    w2_d = nc.dram_tensor("w2", [DOUT, DOUT], F32, kind="ExternalInput").ap()
    b2_d = nc.dram_tensor("b2", [DOUT], F32, kind="ExternalInput").ap()
    gg_d = nc.dram_tensor("gg", [2, 3, LSH], F32, kind="ExternalInput").ap()
    sel_d = nc.dram_tensor(
        "sel", [NCORES, NCHUNK, 128], F32, kind="ExternalInput"
    ).ap()
    out_d = nc.dram_tensor("out", [NB, DOUT], F32, kind="ExternalOutput").ap()

    # collective bounce buffers (internal DRAM; output must be Shared)
    pos_in_d = nc.dram_tensor("pos_in", [1, LSH], F32).ap()
    pos_gather_d = nc.dram_tensor(
        "pos_gather", [NCORES, LSH], F32, addr_space="Shared"
    ).ap()

    with tile.TileContext(nc) as tc:
        with (
            tc.tile_pool(name="const", bufs=1) as cpool,
            tc.tile_pool(name="state", bufs=1) as spool,
        ):
            # live for the whole kernel
            q_rep = cpool.tile([128, 2, 128], F32R)  # q' replicated along free
            ones_row = cpool.tile([1, 128], F32R)
            vwT_sb = cpool.tile([128, 2, DOUT], F32)  # [d%128, d//128, o]
            vb_sb = cpool.tile([128, 2], F32)
            c_rep = cpool.tile([128, 1], F32)  # (k_b+b2)@query/16 replicated
            sel_sb = cpool.tile([NCORES, NCHUNK, 128], F32R)
            pos_all = cpool.tile([NCORES, LSH], F32R)  # gathered pos table
            # per-batch accumulator tiles so batch n's final reduction only
            # waits on batch n's last unit (tile-granular dependencies)
            sexp_n0 = spool.tile([128, 2 * NCHUNK], F32)
            sexp_n1 = spool.tile([128, 2 * NCHUNK], F32)
            wpart_n0 = spool.tile([128, 2, NCHUNK], F32)
            wpart_n1 = spool.tile([128, 2, NCHUNK], F32)
            sexp_by_n = (sexp_n0, sexp_n1)
            wpart_by_n = (wpart_n0, wpart_n1)

            def emit_unit(c8, n, pspool, pos_stat, pos_mov):
                """One (chunk, batch) unit: DMA, scores, exp, fused mul-reduce.

                pos_stat/pos_mov give the stationary AP and moving-slice
                factory for the pos-add matmul (local ones-broadcast for
                chunk 0, one-hot selector over the gathered table otherwise).
                The kb/b2 constant enters via the Exp bias, not the scores.
                """
                sexp_sb = sexp_by_n[n]
                wpart_sb = wpart_by_n[n]
                idx = c8
                x_n = x_d[n].rearrange("(dh p) l -> p dh l", p=128)
                csl = slice(c8 * LC, (c8 + 1) * LC)
                x0 = xpool0.tile([128, LC], F32, tag="x0")
                x1 = xpool1.tile([128, LC], F32, tag="x1")
                nc.sync.dma_start(_r(x0[:]), _r(x_n[:, 0, csl]))
                nc.sync.dma_start(_r(x1[:]), _r(x_n[:, 1, csl]))
                xs = (x0, x1)
                ps0 = pspool.tile([128, LC // 2], F32, tag="s")
                ps1 = pspool.tile([128, LC // 2], F32, tag="s")
                pss = (ps0, ps1)
                for dh in range(2):  # one stationary per dh group
                    for hs in range(2):
                        for s2 in range(2):
                            sl = slice(
                                hs * 1024 + s2 * 512, hs * 1024 + (s2 + 1) * 512
                            )
                            nc.tensor.matmul(
                                pss[hs][:, s2 * 512 : (s2 + 1) * 512],
                                q_rep[:, dh, :],
                                _r(xs[dh][:, sl]),
                                start=(dh == 0),
                                stop=False,
                            )
                for hs in range(2):  # pos stationary once, closes all groups
                    for s2 in range(2):
                        lo = hs * 1024 + s2 * 512
                        nc.tensor.matmul(
                            pss[hs][:, s2 * 512 : (s2 + 1) * 512],
                            pos_stat,
                            pos_mov(lo),
                            start=False,
                            stop=True,
                        )
                u_t = upool.tile([128, LC], F32, tag="u")
                for hs in range(2):
                    nc.scalar.activation(
                        u_t[:, hs * 1024 : (hs + 1) * 1024], pss[hs][:], AF.Exp,
                        bias=c_rep[:],
                        accum_out=sexp_sb[:, 2 * idx + hs : 2 * idx + hs + 1],
                    )
                scr = scrpool.tile([128, LC], F32, tag="scr")
                for dh in range(2):
                    nc.vector.affine_mul_reduce(
                        out=scr[:],
                        accum_out=wpart_sb[:, dh, idx : idx + 1],
                        in0=xs[dh][:],
                        in1=u_t[:],
                        scale=1.0,
                        bias=0.0,
                    )

            with (
                tc.tile_pool(name="xp0", bufs=5) as xpool0,
                tc.tile_pool(name="xp1", bufs=5) as xpool1,
                tc.tile_pool(name="up", bufs=2) as upool,
                tc.tile_pool(name="scr", bufs=2) as scrpool,
                tc.tile_pool(name="htp", bufs=1) as htpool,
                tc.tile_pool(name="ggp", bufs=1) as ggpool,
                tc.tile_pool(name="pre", bufs=1) as ppool,
            ):
                # ---- constant loads --------------------------------------
                # All on the sync HWDGE queue, in front of the x-chunk DMAs
                # (FIFO per queue), so the prologue chain isn't starved by
                # the bulk x stream.
                wrT_sb = ppool.tile([2, 128], F32)  # [k, f]
                nc.sync.dma_start(_r(wrT_sb[:]), _r(wr_d.rearrange("f k -> k f")))
                w1_sb = ppool.tile([128, 2, DOUT], F32)  # [j%128, j//128, f]
                nc.sync.dma_start(
                    w1_sb[:], w1_d.rearrange("(jh p) f -> p jh f", p=128)
                )
                b1_sb = ppool.tile([128, 2], F32)
                nc.sync.dma_start(b1_sb[:], b1_d.rearrange("(jh p) -> p jh", p=128))
                q_sb = ppool.tile([128, 2], F32)  # query as columns
                nc.sync.dma_start(q_sb[:], query_d.rearrange("(oh p) -> p oh", p=128))
                kw_sb = ppool.tile([128, 2, D], F32)  # [o%128, o//128, d]
                nc.sync.dma_start(
                    kw_sb[:], kw_d.rearrange("(oh p) d -> p oh d", p=128)
                )
                w2_sb = ppool.tile([128, 2, DOUT], F32)  # [o%128, o//128, j]
                nc.sync.dma_start(
                    w2_sb[:], w2_d.rearrange("(oh p) j -> p oh j", p=128)
                )
                kb_sb = ppool.tile([128, 2], F32)
                nc.sync.dma_start(kb_sb[:], kb_d.rearrange("(oh p) -> p oh", p=128))
                b2_sb = ppool.tile([128, 2], F32)
                nc.sync.dma_start(b2_sb[:], b2_d.rearrange("(oh p) -> p oh", p=128))
                # gg (own range) + sel ride the idle scalar queue
                gg_t = ggpool.tile([2, LSH], F32, tag="gg")
                nc.scalar.dma_start(_r(gg_t[:]), _r(gg_d[:, 0, :]))
                nc.scalar.dma_start(_r(sel_sb[:]), _r(sel_d))

                ident_sb = ppool.tile([128, 128], F32)
                make_identity(nc, ident_sb[:])
                ones_tile = ppool.tile([128, 128], F32)
                nc.vector.memset(ones_tile[:], 1.0)
                halfpi_sb = ppool.tile([128, 1], F32)
                nc.vector.memset(halfpi_sb[:], HALF_PI)

                # small constant ops on DVE: keeps the Act engine's function
                # table untouched until the first real Sin (table loads cost
                # 1.3 us per function switch).
                qs_sb = ppool.tile([128, 2], F32)  # query / sqrt(DOUT)
                nc.vector.tensor_scalar_mul(qs_sb[:], q_sb[:], INV_SQRT_D)
                nc.vector.memset(ones_row[:].bitcast(F32), 1.0)

                w1T_sb = ppool.tile([128, 2, DOUT], F32R)  # [f, fh, j] / 16
                w2q_col = ppool.tile([128, 2], F32R)
                kb2_sb = ppool.tile([128, 2], F32)

                nc.vector.tensor_tensor(
                    out=kb2_sb[:], in0=kb_sb[:], in1=b2_sb[:], op=OP.add
                )

                # ---- own pos range + AllGather + chunk-0 units ---------------
                def emit_pos_proj(gg_r):
                    """proj + cos/sin for one l-range; returns (cos, sin).

                    cos/sin borrow x-pool ring slots (same shape + tag): they
                    are dead once the hidden stage reads them, so the steady
                    phase still sees the full x runway.
                    """
                    HB = LSH // 2  # 1024: sub-range granularity
                    cos_sb = ppool.tile([128, LSH], F32, tag="cos")
                    sin_sb = ppool.tile([128, LSH], F32, tag="sin")
                    ps_rb0 = psR.tile([128, HB], F32, tag="rb")
                    ps_rb1 = psR.tile([128, HB], F32, tag="rb")
                    ps_rb = (ps_rb0, ps_rb1)
                    for sb2 in range(2):
                        for s in range(2):
                            sl = slice(
                                sb2 * HB + s * 512, sb2 * HB + (s + 1) * 512
                            )
                            nc.tensor.matmul(
                                ps_rb[sb2][:, s * 512 : (s + 1) * 512],
                                _r(wrT_sb[:]), gg_r[:, sl],
                                start=True, stop=True,
                            )
                    for sb2 in range(2):
                        hsl = slice(sb2 * HB, (sb2 + 1) * HB)
                        nc.scalar.activation(
                            _r(cos_sb[:, hsl]), ps_rb[sb2][:], AF.Sin,
                            bias=halfpi_sb[:],
                        )
                        nc.scalar.activation(
                            _r(sin_sb[:, hsl]), ps_rb[sb2][:], AF.Sin
                        )
                    return cos_sb, sin_sb

                def emit_pos_tail(cos_sb, sin_sb, pos_row):
                    """hidden + gelu + output row for one l-range."""
                    HB = LSH // 2
                    hTb = htpool.tile([128, 2, LSH], F32, tag="hT")
                    hT_sb = (hTb[:, 0], hTb[:, 1])
                    for jh in range(2):
                        ps_h0 = psR.tile([128, HB], F32, tag="rb")
                        ps_h1 = psR.tile([128, HB], F32, tag="rb")
                        ps_h = (ps_h0, ps_h1)
                        for fh, src_sb in ((0, cos_sb), (1, sin_sb)):
                            for sb2 in range(2):
                                for s in range(2):
                                    sl = slice(
                                        sb2 * HB + s * 512,
                                        sb2 * HB + (s + 1) * 512,
                                    )
                                    nc.tensor.matmul(
                                        ps_h[sb2][:, s * 512 : (s + 1) * 512],
                                        w1T_sb[:, fh, jh * 128 : (jh + 1) * 128],
                                        _r(src_sb[:, sl]),
                                        start=(fh == 0),
                                        stop=(fh == 1),
                                    )
                        for sb2 in range(2):
                            hsl = slice(sb2 * HB, (sb2 + 1) * HB)
                            nc.scalar.activation(
                                _r(hT_sb[jh][:, hsl]), ps_h[sb2][:],
                                AF.Gelu_apprx_tanh,
                                bias=b1_sb[:, jh : jh + 1],
                            )
                    ps_pos0 = psR.tile([128, HB], F32, tag="rb")
                    ps_pos1 = psR.tile([128, HB], F32, tag="rb")
                    ps_pos = (ps_pos0, ps_pos1)
                    for jh in range(2):
                        for sb2 in range(2):
                            for s in range(2):
                                sl = slice(
                                    sb2 * HB + s * 512, sb2 * HB + (s + 1) * 512
                                )
                                nc.tensor.matmul(
                                    ps_pos[sb2][0:1, s * 512 : (s + 1) * 512],
                                    w2q_col[:, jh : jh + 1],
                                    _r(hT_sb[jh][:, sl]),
                                    start=(jh == 0),
                                    stop=(jh == 1),
                                )
                    # raw pos row (the kb/b2 constant is applied as Exp bias)
                    for sb2 in range(2):
                        nc.vector.tensor_copy(
                            _r(pos_row[0:1, sb2 * HB : (sb2 + 1) * HB]),
                            ps_pos[sb2][0:1, :].bitcast(F32R),
                        )

                with (
                    tc.tile_pool(name="psR", bufs=2, space="PSUM") as psR,
                    tc.tile_pool(name="psM", bufs=2, space="PSUM") as psM,
                ):
                    # j0 proj + cos/sin first: needs only wrT + gg, so the
                    # Act engine starts the pos chain as early as possible
                    with tc.high_priority(offset=2000):
                        cos0, sin0 = emit_pos_proj(_r(gg_t[:]))

                    # w1T transposes: j0's hidden stage needs them.
                    # All prologue PSUM uses slice the standard unit-sized
                    # tile (mixed shapes/tags would inflate the pool).
                    for ah in range(2):
                        for bh in range(2):
                            ps_t = psM.tile([128, LC // 2], F32, tag="s")
                            nc.tensor.transpose(
                                ps_t[:, 0:128],
                                w1_sb[:, ah, bh * 128 : (bh + 1) * 128],
                                ident_sb[:],
                            )
                            nc.vector.tensor_scalar_mul(
                                w1T_sb[:, bh, ah * 128 : (ah + 1) * 128],
                                ps_t[:, 0:128],
                                INV_SQRT_D,
                            )

                    # w2q and c next: they gate the tail of j0's pos
                    # stage and MUST precede it in emission order
                    # (engines execute in order; a consumer emitted
                    # before its producer wedges the whole engine).
                    for jh in range(2):
                        ps_qt = psM.tile([128, LC // 2], F32, tag="s")
                        ps_q = ps_qt[:, 0:1]
                        for oh in range(2):
                            nc.tensor.matmul(
                                ps_q,
                                w2_sb[:, oh, jh * 128 : (jh + 1) * 128],
                                qs_sb[:, oh : oh + 1],
                                start=(oh == 0),
                                stop=(oh == 1),
                            )
                        nc.vector.tensor_copy(w2q_col[:, jh : jh + 1], ps_q)
                    # c replicated over partitions: stationary kb2 replicated
                    # along free via tensor_scalar, then a [K,128]x[K,1] mm.
                    ps_ct = psM.tile([128, LC // 2], F32, tag="s")
                    ps_c = ps_ct[:, 0:1]
                    for oh in range(2):
                        kb2_rep = ppool.tile([128, 128], F32, tag="kb2rep")
                        nc.vector.tensor_scalar_mul(
                            kb2_rep[:], ones_tile[:], kb2_sb[:, oh : oh + 1]
                        )
                        nc.tensor.matmul(
                            ps_c,
                            kb2_rep[:],
                            qs_sb[:, oh : oh + 1],
                            start=(oh == 0),
                            stop=(oh == 1),
                        )
                    nc.vector.tensor_copy(c_rep[:], ps_c)

                    # j0 tail + AllGather dispatch: the collective's ~30 us
                    # framework latency makes this the kernel's critical path
                    pos_own = ppool.tile([1, LC], F32, tag="possh")
                    with tc.high_priority(offset=2000):
                        emit_pos_tail(cos0, sin0, pos_own)
                        # gpsimd SWDGE: the sync FIFO is jammed with x-chunk
                        # descgens and the Act FIFO must keep draining Exps —
                        # either would stall this data-dependent descgen (and
                        # with it the collective trigger behind it).
                        nc.gpsimd.dma_start(pos_in_d, pos_own[:].bitcast(F32))
                        nc.gpsimd.collective_compute(
                            "AllGather",
                            OP.bypass,
                            replica_groups=[list(range(NCORES))],
                            ins=[pos_in_d],
                            outs=[pos_gather_d],
                        )

                    # q' gates only the units' score matmuls
                    for dh in range(2):
                        ps_qt = psM.tile([128, LC // 2], F32, tag="s")
                        ps_q = ps_qt[:, 0:1]
                        for oh in range(2):
                            nc.tensor.matmul(
                                ps_q,
                                kw_sb[:, oh, dh * 128 : (dh + 1) * 128],
                                qs_sb[:, oh : oh + 1],
                                start=(oh == 0),
                                stop=(oh == 1),
                            )
                        qcol = ppool.tile([128, 1], F32, tag="qcol")
                        nc.vector.tensor_copy(qcol[:], ps_q)
                        nc.vector.tensor_scalar_mul(
                            q_rep[:, dh, :], ones_tile[:], qcol[:]
                        )

                    # chunk 0 == own range: consume the local pos row
                    for n in range(NB):
                        emit_unit(
                            0, n, psM,
                            ones_row[:],
                            lambda lo: _r(pos_own[0:1, lo : lo + 512]),
                        )

                    # bridge ranges (own+1, own+2) computed locally for
                    # chunks 1-2, so the pipeline rides through the
                    # collective's latency. Both ranges' stages are emitted
                    # back-to-back BEFORE the bridge units' Exps: the Act
                    # engine runs in order, so this caps the 1.3 us
                    # activation-table reloads at one per function.
                    pos_r1 = ppool.tile([1, LC], F32, tag="posr1")
                    gg_t1 = ggpool.tile([2, LSH], F32, tag="gg")
                    nc.gpsimd.dma_start(_r(gg_t1[:]), _r(gg_d[:, 1, :]))
                    cos1, sin1 = emit_pos_proj(_r(gg_t1[:]))
                    emit_pos_tail(cos1, sin1, pos_r1)

                    pos_r2 = ppool.tile([1, LC], F32, tag="posr2")
                    gg_t2 = ggpool.tile([2, LSH], F32, tag="gg")
                    nc.gpsimd.dma_start(_r(gg_t2[:]), _r(gg_d[:, 2, :]))
                    cos2, sin2 = emit_pos_proj(_r(gg_t2[:]))
                    emit_pos_tail(cos2, sin2, pos_r2)

                    for n in range(NB):
                        emit_unit(
                            1, n, psM,
                            ones_row[:],
                            lambda lo: _r(pos_r1[0:1, lo : lo + 512]),
                        )
                    for n in range(NB):
                        emit_unit(
                            2, n, psM,
                            ones_row[:],
                            lambda lo: _r(pos_r2[0:1, lo : lo + 512]),
                        )

                # ---- V constants, overlapping the main loop ------------------
                with tc.tile_pool(name="psV", bufs=2, space="PSUM") as psV:
                    vw_sb = ppool.tile([128, 2, D], F32)
                    nc.scalar.dma_start(
                        vw_sb[:], vw_d.rearrange("(oh p) d -> p oh d", p=128)
                    )
                    nc.scalar.dma_start(
                        vb_sb[:], vb_d.rearrange("(oh p) -> p oh", p=128)
                    )
                    for ah in range(2):
                        for bh in range(2):
                            ps_t2 = psV.tile([128, 128], F32, tag="tr")
                            nc.tensor.transpose(
                                ps_t2[:],
                                vw_sb[:, ah, bh * 128 : (bh + 1) * 128],
                                ident_sb[:],
                            )
                            nc.vector.tensor_copy(
                                vwT_sb[:, bh, ah * 128 : (ah + 1) * 128], ps_t2[:]
                            )

                # ---- remaining chunks select from the gathered pos table -----
                # psR is closed here, so a 4-deep PSUM pool (all 8 banks)
                # lets the PE run ahead of the Exp drain without stalling.
                # gpsimd queue: this load depends on the collective, and on
                # the Act queue its descgen would wedge the Exp stream (the
                # scheduler may order it between earlier activations)
                nc.gpsimd.dma_start(_r(pos_all[:]), _r(pos_gather_d))

                def emit_final(n, pspool):
                    """normalize + V projection + store for one batch."""
                    s_col = fpool.tile([128, 1], F32, tag="scol")
                    nc.vector.tensor_reduce(
                        s_col[:], sexp_by_n[n][:], mybir.AxisListType.X, OP.add
                    )
                    srec = fpool.tile([128, 1], F32, tag="srec")
                    nc.vector.reciprocal(srec[:], s_col[:])
                    wn = fpool.tile([128, 2], F32, tag="wn")
                    for dh in range(2):
                        wsum = fpool.tile([128, 1], F32, tag="wsum")
                        nc.vector.tensor_reduce(
                            wsum[:], wpart_by_n[n][:, dh, :],
                            mybir.AxisListType.X, OP.add,
                        )
                        nc.vector.tensor_scalar_mul(
                            wn[:, dh : dh + 1], wsum[:], srec[:]
                        )
                    for oh in range(2):
                        ps_ot = pspool.tile([128, LC // 2], F32, tag="s")
                        ps_o = ps_ot[:, 0:1]
                        for dh in range(2):
                            nc.tensor.matmul(
                                ps_o,
                                vwT_sb[:, dh, oh * 128 : (oh + 1) * 128],
                                wn[:, dh : dh + 1],
                                start=(dh == 0),
                                stop=(dh == 1),
                            )
                        o_sb = fpool.tile([128, 1], F32, tag="osb")
                        nc.vector.tensor_scalar_add(
                            o_sb[:], ps_o, vb_sb[:, oh : oh + 1]
                        )
                        nc.sync.dma_start(
                            out_d[n : n + 1, oh * 128 : (oh + 1) * 128], o_sb[:]
                        )

                with (
                    tc.tile_pool(name="psM2", bufs=4, space="PSUM") as psM2,
                    tc.tile_pool(name="fin", bufs=4) as fpool,
                ):
                    for c8 in range(3, NCHUNK):
                        for n in range(NB):
                            emit_unit(
                                c8, n, psM2,
                                sel_sb[:, c8, :],
                                lambda lo: pos_all[:, lo : lo + 512],
                            )
                            # batch n's accumulators are complete after its
                            # last chunk: finish it while the other drains
                            if c8 == NCHUNK - 1:
                                emit_final(n, psM2)

    nc.compile()
    return nc


_NC_CACHE = []


def _get_nc():
    if not _NC_CACHE:
        _NC_CACHE.append(build_program())
    return _NC_CACHE[0]


def _grid_rows():
    """[gy; gx] rows of the normalized meshgrid, flattened to length L."""
    ys = np.linspace(-1.0, 1.0, H, dtype=np.float64)
    xs = np.linspace(-1.0, 1.0, W, dtype=np.float64)
    gy = np.repeat(ys, W)
    gx = np.tile(xs, H)
    return np.stack([gy, gx]).astype(np.float32)  # [2, L]


def make_in_maps(inputs):
    x = np.ascontiguousarray(inputs["x"], dtype=np.float32).reshape(N, D, L)
    gg = _grid_rows()
    small = {
        k: np.ascontiguousarray(np.asarray(inputs[k], dtype=np.float32))
        for k in ("query", "k_w", "k_b", "v_w", "v_b", "Wr", "w1", "b1", "w2", "b2")
    }
    in_maps = []
    for c in range(NCORES):
        m = dict(small)
        # rotate the l-chunks so chunk j holds range (c+j)%NCHUNK
        xc = x[c * NB : (c + 1) * NB].reshape(NB, D, NCHUNK, LC)
        m["x_sh"] = np.ascontiguousarray(
            np.roll(xc, -c, axis=2).reshape(NB, D, L)
        )
        ggc = np.stack(
            [
                gg[:, r * LSH : (r + 1) * LSH]
                for r in (c, (c + 1) % NCORES, (c + 2) % NCORES)
            ],
            axis=1,
        )  # [2, 3, LSH]
        m["gg"] = np.ascontiguousarray(ggc)
        sel = np.zeros((NCORES, NCHUNK, 128), dtype=np.float32)
        for j in range(NCHUNK):
            sel[(c + j) % NCHUNK, j, :] = 1.0
        m["sel"] = sel
        in_maps.append(m)
    return in_maps


def run(inputs, trace=False):
    nc = _get_nc()
    res = run_bass_kernel_spmd(
        nc, make_in_maps(inputs), core_ids=list(range(NCORES)), trace=trace
    )
    out = np.concatenate([res.results[c]["out"] for c in range(NCORES)], axis=0)
    return out.astype(np.float32), res


def kernel(**inputs) -> np.ndarray:
    out, _ = run(inputs, trace=False)
    return out

